# revision 22
# baseline (speedup 1.0000x reference)
"""Trainium2 Bass kernel for the dense MLP:

    h1  = relu(x @ W1.T + b1)         x:[B,D] W1:[HID,D]
    out = [x, h1] @ W2.T + b2         W2:[OUT, D+HID]

Strategy: data-parallel over the batch across 8 NeuronCores (512 rows
each), weights replicated.  Matmuls run in bf16 with fp32 PSUM
accumulation, EXCEPT the first `ktf` k-tiles of layer 1 which run in
fp8-e4m3 with perf_mode=DoubleRow (2 k-planes per PE cycle, measured at
full 2x).  The fp8 fraction is tuned so the end-to-end relative error
stays ~1.9e-2, under the 2e-2 gate (each fp8 k-plane contributes
quantization noise 2*eps^2, eps=0.0265 for e4m3 on gaussian data; the
error budget is ~4.4x more cycle-efficient spent in layer 1 than in
layer 2, so layer 2 stays bf16).

Scale folding keeps the device program free of extra ops: W1 is scaled
by 8 before quantization (sigma 0.125, clear of e4m3 denormals), b1 by
8, so h1 is stored as 8*relu(...); the h-columns of W2 are divided by
8 host-side (exact in bf16).

Phase order is chosen to dissolve the startup DMA crunch: layer 1 needs
only xq (0.75MB) + xt k-tiles 12..31 (2.5MB) + one W1 tile to start, so
it begins ~4us in; the layer-2 x-part (which needs the rest of xt and
8MB of W2) runs at the END as phase 3, where DMA has had the whole
kernel to stream.  Per core:

  warmup : a few dummy matmuls on a zeroed tile warm the PE clock (HAM)
           while the first DMAs land.
  phase 1: h1T tiles [128h x 512b]: 6 DoubleRow fp8 matmuls (k-tiles
           0..11 paired) + 20 bf16 matmuls (k-tiles 12..31) into one
           PSUM tile, then bias+ReLU via DVE into resident SBUF.
  phase 2: h-part of out for both 500-col output halves: [128b x 500o]
           PSUM tiles accumulated over k-tiles 32..159 (8 banks live).
  phase 3: x-part (k-tiles 0..31) accumulated on top; half 0 evicts
           while half 1's matmuls still run; half 1 finishes bt-major
           so evictions overlap the last matmuls.

Host side pre-transposes/reorders x, W1, W2 into partition-major DRAM
layouts (multi-KB contiguous per-partition lines, so HWDGE packets are
large) and adds b2 to the gathered output.
"""

import numpy as np
import ml_dtypes

import concourse.bacc as bacc
import concourse.mybir as mybir
import concourse.tile as tile
from concourse.bass_utils import run_bass_kernel_spmd

B, D, HID, OUT = 4096, 4096, 16384, 1000
NCORES = 8
BC = B // NCORES  # rows of x per core
KTF = 12          # k-tiles of layer 1 in fp8 DoubleRow (of D//128 = 32)
S1 = 8.0          # W1/b1 pre-scale folded out via W2 h-columns

bf16 = mybir.dt.bfloat16
f8 = mybir.dt.float8e4
f32 = mybir.dt.float32
nbf = ml_dtypes.bfloat16
nf8 = ml_dtypes.float8_e4m3

_cache = {}


def build(d=D, hid=HID, out_n=OUT, bc=BC, ktf=KTF, w1_bufs=3, w2_bufs=4,
          ps1_bufs=4, kb=4, n_w2_prefetch=3, n_warm=5):
    """Build + compile the per-core Bass program. Returns the Bacc."""
    kt1 = d // 128          # k-tiles in layer 1
    nh = hid // 128         # h-tiles
    kt2 = (d + hid) // 128  # k-tiles in layer 2
    nb = bc // 128          # b-tiles per core
    ocs = out_n // 2        # output split in two halves (<=512 each)
    assert ocs <= 512
    assert ktf % 2 == 0
    n_w2_prefetch = min(n_w2_prefetch, w2_bufs - 2, (kt2 - kt1) // kb)

    nc = bacc.Bacc("TRN2", target_bir_lowering=False, debug=False,
                   num_devices=NCORES)

    # partition-major DRAM layouts: per-partition lines are multi-KB
    # contiguous, so HWDGE packets are 4-32KB instead of 1KB
    XT = nc.dram_tensor("xt", [128, kt1, bc], bf16, kind="ExternalInput")
    XQ = nc.dram_tensor("xq", [128, ktf, bc], f8, kind="ExternalInput")
    W1F = nc.dram_tensor("w1f", [nh, 128, ktf, 128], f8, kind="ExternalInput")
    W1B = nc.dram_tensor("w1b", [nh, 128, (kt1 - ktf) * 128], bf16,
                         kind="ExternalInput")
    W2A = nc.dram_tensor("w2a", [128, kt2, ocs], bf16, kind="ExternalInput")
    W2B = nc.dram_tensor("w2b", [128, kt2, out_n - ocs], bf16,
                         kind="ExternalInput")
    B1R = nc.dram_tensor("b1r", [128, nh], f32, kind="ExternalInput")
    OUTT = nc.dram_tensor("out", [bc, out_n], f32, kind="ExternalOutput")

    add_op = mybir.AluOpType.add
    max_op = mybir.AluOpType.max
    dr = mybir.MatmulPerfMode.DoubleRow
    # two independent HWDGE rings (qSyncDynamicHW / qScalarDynamicHW)
    rings = [nc.sync, nc.scalar]

    def w2_dma(ring, w2_t, kt0, oh):
        src = W2A if oh == 0 else W2B
        ring.dma_start(w2_t[:], src.ap()[:, kt0:kt0 + kb, :])

    with tile.TileContext(nc) as tc:
        with (
            tc.tile_pool(name="persist", bufs=1) as persist,
            tc.tile_pool(name="w2", bufs=w2_bufs) as w2p,
        ):
            xt_sb = persist.tile([128, kt1, bc], bf16, tag="xt")
            xq_sb = persist.tile([128, ktf, bc], f8, tag="xq")
            h1_sb = persist.tile([128, nh, bc], bf16, tag="h1")
            b1_sb = persist.tile([128, nh], f32, tag="b1")
            warm_sb = persist.tile([128, bc], bf16, tag="warm")

            w2_pre = []

            with (
                tc.tile_pool(name="w1f", bufs=w1_bufs) as w1fp,
                tc.tile_pool(name="w1b", bufs=w1_bufs) as w1bp,
                tc.tile_pool(name="ps1", bufs=ps1_bufs,
                             space="PSUM") as ps1,
            ):
                # PE warmup: dummy matmuls on a zeroed scratch tile fill
                # the DMA cold-start window so the HAM clock ramp runs
                # on throwaway work (DVE does the memset immediately)
                nc.vector.memset(warm_sb[:], 0.0)
                warm_ps = ps1.tile([128, bc], f32)
                for _ in range(n_warm):
                    nc.tensor.matmul(warm_ps[:], warm_sb[:, 0:128],
                                     warm_sb[:], start=True, stop=True)

                n_lead = min(3, nh, w1_bufs)
                w1f_lead = [w1fp.tile([128, ktf, 128], f8, name="w1f_t")
                            for _ in range(n_lead)]
                w1b_lead = [w1bp.tile([128, (kt1 - ktf) * 128], bf16,
                                      name="w1b_t")
                            for _ in range(n_lead)]
                # sync ring: xq (gates the first real matmuls), then the
                # bf16 x.T tiles phase 1 needs (12..31) in chunks, then
                # b1.  x.T tiles 0..11 are only needed by phase 3 and
                # are emitted at the end of phase 1.  scalar ring: the
                # W1 lead tiles (first h-tiles' weights).
                nc.sync.dma_start(xq_sb[:], XQ.ap()[:])
                nc.sync.dma_start(b1_sb[:], B1R.ap()[:])
                kt0 = ktf
                for n in (4, 4, 8, 4):
                    nc.sync.dma_start(xt_sb[:, kt0:kt0 + n, :],
                                      XT.ap()[:, kt0:kt0 + n, :])
                    kt0 += n
                assert kt0 == kt1
                for hi in range(n_lead):
                    nc.scalar.dma_start(w1f_lead[hi][:], W1F.ap()[hi])
                    nc.scalar.dma_start(w1b_lead[hi][:], W1B.ap()[hi])

                # ---- phase 1: h1T = relu(fp8/bf16 W1 @ x_c.T + b1) ----
                for hi in range(nh):
                    if hi == min(8, nh - 1):
                        # prefetch the first h-part W2 batches so phase 2
                        # starts instantly at the boundary
                        for i in range(n_w2_prefetch):
                            w2_t = w2p.tile([128, kb, ocs], bf16,
                                            name="w2_t")
                            w2_dma(rings[i % 2], w2_t, kt1 + i * kb, 0)
                            w2_pre.append(w2_t)
                    if hi == 16:
                        # x.T tiles 0..11 (phase-3 lhsT): queue behind
                        # the early W1 stream, far ahead of their use
                        nc.sync.dma_start(xt_sb[:, 0:ktf, :],
                                          XT.ap()[:, 0:ktf, :])
                    if hi < n_lead:
                        w1f_t = w1f_lead[hi]
                        w1b_t = w1b_lead[hi]
                    else:
                        w1f_t = w1fp.tile([128, ktf, 128], f8, name="w1f_t")
                        w1b_t = w1bp.tile([128, (kt1 - ktf) * 128], bf16,
                                          name="w1b_t")
                        rings[hi % 2].dma_start(w1f_t[:], W1F.ap()[hi])
                        rings[hi % 2].dma_start(w1b_t[:], W1B.ap()[hi])
                    acc = ps1.tile([128, bc], f32)
                    # fp8 DoubleRow over paired k-tiles 0..ktf-1
                    for kp in range(ktf // 2):
                        nc.tensor.matmul(
                            acc[:],
                            w1f_t[:, 2 * kp:2 * kp + 2, :],
                            xq_sb[:, 2 * kp:2 * kp + 2, :],
                            start=(kp == 0), stop=False,
                            perf_mode=dr,
                        )
                    # bf16 over k-tiles ktf..kt1-1
                    for kt in range(ktf, kt1):
                        ko = kt - ktf
                        nc.tensor.matmul(
                            acc[:],
                            w1b_t[:, ko * 128:(ko + 1) * 128],
                            xt_sb[:, kt, :],
                            start=False, stop=(kt == kt1 - 1),
                        )
                    # fused relu(acc + b1) on DVE, keeping ScalarE free
                    # to pump the weight-stream DMA ring
                    nc.vector.tensor_scalar(
                        h1_sb[:, hi, :], acc[:],
                        b1_sb[:, hi:hi + 1], 0.0, add_op, max_op)

            # ---- phases 2+3: out = concat @ W2 (bf16), 8 PSUM banks ----
            with (
                tc.tile_pool(name="psacc", bufs=1, space="PSUM") as psacc,
                tc.tile_pool(name="outp", bufs=2) as outp,
            ):
                accs = [[psacc.tile([128, ocs], f32, tag=f"a{oh}_{bt}",
                                    name=f"acc2_{oh}_{bt}")
                         for bt in range(nb)] for oh in (0, 1)]

                def evict_one(acc, bt, oh, chunks=1):
                    out_t = outp.tile([128, ocs], f32)
                    # split across DVE and ACT so evictions drain in
                    # parallel; chunks>1 pipelines copy->DMA for the
                    # kernel-final eviction
                    cs = ocs // chunks
                    for c in range(chunks):
                        sl = slice(c * cs, (c + 1) * cs)
                        if (bt + c) % 2 == 0:
                            nc.vector.tensor_copy(out_t[:, sl], acc[:, sl])
                        else:
                            nc.scalar.activation(
                                out_t[:, sl], acc[:, sl],
                                mybir.ActivationFunctionType.Copy)
                        rings[(bt + c) % 2].dma_start(
                            OUTT.ap()[bt * 128:(bt + 1) * 128,
                                      oh * ocs + c * cs:
                                      oh * ocs + (c + 1) * cs],
                            out_t[:, sl])

                # phase 2: h-part for both output halves
                for oh in (0, 1):
                    for bi, kt0 in enumerate(range(kt1, kt2, kb)):
                        if oh == 0 and bi < n_w2_prefetch:
                            w2_t = w2_pre[bi]
                        else:
                            w2_t = w2p.tile([128, kb, ocs], bf16,
                                            name="w2_t")
                            w2_dma(rings[bi % 2], w2_t, kt0, oh)
                        for j in range(kb):
                            kt = kt0 + j
                            for bt in range(nb):
                                nc.tensor.matmul(
                                    accs[oh][bt][:],
                                    h1_sb[:, kt - kt1,
                                          bt * 128:bt * 128 + 128],
                                    w2_t[:, j, :],
                                    start=(kt == kt1), stop=False)

                # phase 3: x-part.  half 0 fully, evict it (overlaps
                # half 1's matmuls), then half 1 with the last two
                # batches bt-major so evictions overlap the tail.
                for bi, kt0 in enumerate(range(0, kt1, kb)):
                    w2_t = w2p.tile([128, kb, ocs], bf16, name="w2_t")
                    w2_dma(rings[bi % 2], w2_t, kt0, 0)
                    for j in range(kb):
                        kt = kt0 + j
                        for bt in range(nb):
                            nc.tensor.matmul(
                                accs[0][bt][:],
                                xt_sb[:, kt, bt * 128:bt * 128 + 128],
                                w2_t[:, j, :],
                                start=False, stop=(kt == kt1 - 1))
                for bt in range(nb):
                    evict_one(accs[0][bt], bt, 0)

                tail0 = kt1 - 2 * kb
                for bi, kt0 in enumerate(range(0, tail0, kb)):
                    w2_t = w2p.tile([128, kb, ocs], bf16, name="w2_t")
                    w2_dma(rings[bi % 2], w2_t, kt0, 1)
                    for j in range(kb):
                        kt = kt0 + j
                        for bt in range(nb):
                            nc.tensor.matmul(
                                accs[1][bt][:],
                                xt_sb[:, kt, bt * 128:bt * 128 + 128],
                                w2_t[:, j, :],
                                start=False, stop=False)
                w2_ta = w2p.tile([128, kb, ocs], bf16, name="w2_t")
                w2_dma(rings[0], w2_ta, tail0, 1)
                w2_tb = w2p.tile([128, kb, ocs], bf16, name="w2_t")
                w2_dma(rings[1], w2_tb, tail0 + kb, 1)
                for bt in range(nb):
                    for w2x, k0 in ((w2_ta, tail0), (w2_tb, tail0 + kb)):
                        for j in range(kb):
                            kt = k0 + j
                            nc.tensor.matmul(
                                accs[1][bt][:],
                                xt_sb[:, kt, bt * 128:bt * 128 + 128],
                                w2x[:, j, :],
                                start=False, stop=(kt == kt1 - 1))
                    evict_one(accs[1][bt], bt, 1,
                              chunks=(2 if bt == nb - 1 else 1))

    nc.compile()
    return nc


def prep_inputs(x, W1, b1, W2, b2, bc=BC, ktf=KTF):
    """Host-side cast to bf16/fp8 + re-layout so device DMAs are
    contiguous.  Folds the S1 scale: W1,b1 scaled up, W2 h-cols down."""
    d = x.shape[1]
    hid = W1.shape[0]
    out_n = W2.shape[0]
    nh = hid // 128
    kt1 = d // 128
    kt2 = (d + hid) // 128

    w1s = np.asarray(W1, np.float32) * S1
    # [hi, p, kt, h] = S1*W1[hi*128+h, kt*128+p]
    w1_4d = w1s.reshape(nh, 128, kt1, 128).transpose(0, 3, 2, 1)
    w1f = np.ascontiguousarray(w1_4d[:, :, :ktf, :]).astype(nf8)
    w1b = np.ascontiguousarray(w1_4d[:, :, ktf:, :]).astype(nbf) \
        .reshape(nh, 128, (kt1 - ktf) * 128)

    w2s = np.asarray(W2, np.float32).copy()
    w2s[:, d:] /= S1
    w2b = w2s.astype(nbf)
    ocs = out_n // 2
    # W2P[p, kt, o] = W2'[o, kt*128+p]  (partition-major, 4KB lines)
    w2p = w2b.reshape(out_n, kt2, 128).transpose(2, 1, 0)
    w2a = np.ascontiguousarray(w2p[:, :, :ocs])
    w2bb = np.ascontiguousarray(w2p[:, :, ocs:])

    b1r = np.ascontiguousarray(
        (np.asarray(b1, np.float32) * S1).reshape(nh, 128).T)

    xb = np.asarray(x).astype(nbf)
    x8 = np.asarray(x, np.float32).astype(nf8)
    ncores = x.shape[0] // bc
    in_maps = []
    for c in range(ncores):
        # [p, kt, b] partition-major
        xt_c = np.ascontiguousarray(
            xb[c * bc:(c + 1) * bc].T.reshape(kt1, 128, bc)
            .transpose(1, 0, 2))
        xq_c = np.ascontiguousarray(
            x8[c * bc:(c + 1) * bc, :ktf * 128].T.reshape(ktf, 128, bc)
            .transpose(1, 0, 2))
        in_maps.append({"xt": xt_c, "xq": xq_c, "w1f": w1f, "w1b": w1b,
                        "w2a": w2a, "w2b": w2bb, "b1r": b1r})
    return in_maps


def kernel(x, W1, b1, W2, b2):
    x = np.asarray(x)
    W1, b1 = np.asarray(W1), np.asarray(b1)
    W2, b2 = np.asarray(W2), np.asarray(b2)

    if "nc" not in _cache:
        _cache["nc"] = build()
    nc = _cache["nc"]

    in_maps = prep_inputs(x, W1, b1, W2, b2)
    res = run_bass_kernel_spmd(nc, in_maps, core_ids=list(range(NCORES)))
    out = np.concatenate([res.results[c]["out"] for c in range(NCORES)],
                         axis=0)
    return out + np.asarray(b2, np.float32)[None, :]


# revision 24
# speedup vs baseline: 1.0009x; 1.0009x over previous
"""Trainium2 Bass kernel for the dense MLP:

    h1  = relu(x @ W1.T + b1)         x:[B,D] W1:[HID,D]
    out = [x, h1] @ W2.T + b2         W2:[OUT, D+HID]

Strategy: data-parallel over the batch across 8 NeuronCores (512 rows
each), weights replicated.  Matmuls run in bf16 with fp32 PSUM
accumulation, EXCEPT the first `ktf` k-tiles of layer 1 which run in
fp8-e4m3 with perf_mode=DoubleRow (2 k-planes per PE cycle, measured at
full 2x).  The fp8 fraction is tuned so the end-to-end relative error
stays ~1.9e-2, under the 2e-2 gate (each fp8 k-plane contributes
quantization noise 2*eps^2, eps=0.0265 for e4m3 on gaussian data; the
error budget is ~4.4x more cycle-efficient spent in layer 1 than in
layer 2, so layer 2 stays bf16).

Scale folding keeps the device program free of extra ops: W1 is scaled
by 8 before quantization (sigma 0.125, clear of e4m3 denormals), b1 by
8, so h1 is stored as 8*relu(...); the h-columns of W2 are divided by
8 host-side (exact in bf16).

Phase order is chosen to dissolve the startup DMA crunch: layer 1 needs
only xq (0.75MB) + xt k-tiles 12..31 (2.5MB) + one W1 tile to start, so
it begins ~4us in; the layer-2 x-part (which needs the rest of xt and
8MB of W2) runs at the END as phase 3, where DMA has had the whole
kernel to stream.  Per core:

  warmup : a few dummy matmuls on a zeroed tile warm the PE clock (HAM)
           while the first DMAs land.
  phase 1: h1T tiles [128h x 512b]: 6 DoubleRow fp8 matmuls (k-tiles
           0..11 paired) + 20 bf16 matmuls (k-tiles 12..31) into one
           PSUM tile, then bias+ReLU via DVE into resident SBUF.
  phase 2: h-part of out for both 500-col output halves: [128b x 500o]
           PSUM tiles accumulated over k-tiles 32..159 (8 banks live).
  phase 3: x-part (k-tiles 0..31) accumulated on top; half 0 evicts
           while half 1's matmuls still run; half 1 finishes bt-major
           so evictions overlap the last matmuls.

Host side pre-transposes/reorders x, W1, W2 into partition-major DRAM
layouts (multi-KB contiguous per-partition lines, so HWDGE packets are
large) and adds b2 to the gathered output.
"""

import numpy as np
import ml_dtypes

import concourse.bacc as bacc
import concourse.mybir as mybir
import concourse.tile as tile
from concourse.bass_utils import run_bass_kernel_spmd

B, D, HID, OUT = 4096, 4096, 16384, 1000
NCORES = 8
BC = B // NCORES  # rows of x per core
KTF = 12          # k-tiles of layer 1 in fp8 DoubleRow (of D//128 = 32)
S1 = 8.0          # W1/b1 pre-scale folded out via W2 h-columns

bf16 = mybir.dt.bfloat16
f8 = mybir.dt.float8e4
f32 = mybir.dt.float32
nbf = ml_dtypes.bfloat16
nf8 = ml_dtypes.float8_e4m3

_cache = {}


def build(d=D, hid=HID, out_n=OUT, bc=BC, ktf=KTF, w1_bufs=3, w2_bufs=4,
          ps1_bufs=4, kb=4, n_w2_prefetch=3, n_warm=5):
    """Build + compile the per-core Bass program. Returns the Bacc."""
    kt1 = d // 128          # k-tiles in layer 1
    nh = hid // 128         # h-tiles
    kt2 = (d + hid) // 128  # k-tiles in layer 2
    nb = bc // 128          # b-tiles per core
    ocs = out_n // 2        # output split in two halves (<=512 each)
    assert ocs <= 512
    assert ktf % 2 == 0
    n_w2_prefetch = min(n_w2_prefetch, w2_bufs - 2, (kt2 - kt1) // kb)

    nc = bacc.Bacc("TRN2", target_bir_lowering=False, debug=False,
                   num_devices=NCORES)

    # partition-major DRAM layouts: per-partition lines are multi-KB
    # contiguous, so HWDGE packets are 4-32KB instead of 1KB
    XT = nc.dram_tensor("xt", [128, kt1, bc], bf16, kind="ExternalInput")
    XQ = nc.dram_tensor("xq", [128, ktf, bc], f8, kind="ExternalInput")
    W1F = nc.dram_tensor("w1f", [nh, 128, ktf, 128], f8, kind="ExternalInput")
    W1B = nc.dram_tensor("w1b", [nh, 128, (kt1 - ktf) * 128], bf16,
                         kind="ExternalInput")
    W2A = nc.dram_tensor("w2a", [128, kt2, ocs], bf16, kind="ExternalInput")
    W2B = nc.dram_tensor("w2b", [128, kt2, out_n - ocs], bf16,
                         kind="ExternalInput")
    B1R = nc.dram_tensor("b1r", [128, nh], f32, kind="ExternalInput")
    OUTT = nc.dram_tensor("out", [bc, out_n], f32, kind="ExternalOutput")

    add_op = mybir.AluOpType.add
    max_op = mybir.AluOpType.max
    dr = mybir.MatmulPerfMode.DoubleRow
    # two independent HWDGE rings (qSyncDynamicHW / qScalarDynamicHW)
    rings = [nc.sync, nc.scalar]

    def w2_dma(ring, w2_t, kt0, oh):
        src = W2A if oh == 0 else W2B
        ring.dma_start(w2_t[:], src.ap()[:, kt0:kt0 + kb, :])

    with tile.TileContext(nc) as tc:
        with (
            tc.tile_pool(name="persist", bufs=1) as persist,
            tc.tile_pool(name="w2", bufs=w2_bufs) as w2p,
        ):
            xt_sb = persist.tile([128, kt1, bc], bf16, tag="xt")
            xq_sb = persist.tile([128, ktf, bc], f8, tag="xq")
            h1_sb = persist.tile([128, nh, bc], bf16, tag="h1")
            b1_sb = persist.tile([128, nh], f32, tag="b1")
            warm_sb = persist.tile([128, bc], bf16, tag="warm")

            w2_pre = []

            with (
                tc.tile_pool(name="w1f", bufs=w1_bufs) as w1fp,
                tc.tile_pool(name="w1b", bufs=w1_bufs) as w1bp,
                tc.tile_pool(name="ps1", bufs=ps1_bufs,
                             space="PSUM") as ps1,
            ):
                # PE warmup: dummy matmuls on a zeroed scratch tile fill
                # the DMA cold-start window so the HAM clock ramp runs
                # on throwaway work (DVE does the memset immediately)
                nc.vector.memset(warm_sb[:], 0.0)
                warm_ps = ps1.tile([128, bc], f32)
                for _ in range(n_warm):
                    nc.tensor.matmul(warm_ps[:], warm_sb[:, 0:128],
                                     warm_sb[:], start=True, stop=True)

                n_lead = min(3, nh, w1_bufs)
                w1f_lead = [w1fp.tile([128, ktf, 128], f8, name="w1f_t")
                            for _ in range(n_lead)]
                w1b_lead = [w1bp.tile([128, (kt1 - ktf) * 128], bf16,
                                      name="w1b_t")
                            for _ in range(n_lead)]
                # sync ring: xq (gates the first real matmuls), then the
                # bf16 x.T tiles phase 1 needs (12..31) in chunks, then
                # b1.  x.T tiles 0..11 are only needed by phase 3 and
                # are emitted at the end of phase 1.  scalar ring: the
                # W1 lead tiles (first h-tiles' weights).
                nc.sync.dma_start(xq_sb[:], XQ.ap()[:])
                nc.sync.dma_start(b1_sb[:], B1R.ap()[:])
                kt0 = ktf
                for n in (4, 4, 8, 4):
                    nc.sync.dma_start(xt_sb[:, kt0:kt0 + n, :],
                                      XT.ap()[:, kt0:kt0 + n, :])
                    kt0 += n
                assert kt0 == kt1
                for hi in range(n_lead):
                    nc.scalar.dma_start(w1f_lead[hi][:], W1F.ap()[hi])
                    nc.scalar.dma_start(w1b_lead[hi][:], W1B.ap()[hi])

                # ---- phase 1: h1T = relu(fp8/bf16 W1 @ x_c.T + b1) ----
                for hi in range(nh):
                    if hi == min(8, nh - 1):
                        # prefetch the first h-part W2 batches so phase 2
                        # starts instantly at the boundary
                        for i in range(n_w2_prefetch):
                            w2_t = w2p.tile([128, kb, ocs], bf16,
                                            name="w2_t")
                            w2_dma(rings[i % 2], w2_t, kt1 + i * kb, 0)
                            w2_pre.append(w2_t)
                    if hi == 16:
                        # x.T tiles 0..11 (phase-3 lhsT): queue behind
                        # the early W1 stream, far ahead of their use
                        nc.sync.dma_start(xt_sb[:, 0:ktf, :],
                                          XT.ap()[:, 0:ktf, :])
                    if hi < n_lead:
                        w1f_t = w1f_lead[hi]
                        w1b_t = w1b_lead[hi]
                    else:
                        w1f_t = w1fp.tile([128, ktf, 128], f8, name="w1f_t")
                        w1b_t = w1bp.tile([128, (kt1 - ktf) * 128], bf16,
                                          name="w1b_t")
                        rings[hi % 2].dma_start(w1f_t[:], W1F.ap()[hi])
                        rings[hi % 2].dma_start(w1b_t[:], W1B.ap()[hi])
                    acc = ps1.tile([128, bc], f32)
                    # fp8 DoubleRow over paired k-tiles 0..ktf-1
                    for kp in range(ktf // 2):
                        nc.tensor.matmul(
                            acc[:],
                            w1f_t[:, 2 * kp:2 * kp + 2, :],
                            xq_sb[:, 2 * kp:2 * kp + 2, :],
                            start=(kp == 0), stop=False,
                            perf_mode=dr,
                        )
                    # bf16 over k-tiles ktf..kt1-1
                    for kt in range(ktf, kt1):
                        ko = kt - ktf
                        nc.tensor.matmul(
                            acc[:],
                            w1b_t[:, ko * 128:(ko + 1) * 128],
                            xt_sb[:, kt, :],
                            start=False, stop=(kt == kt1 - 1),
                        )
                    # fused relu(acc + b1) on DVE, keeping ScalarE free
                    # to pump the weight-stream DMA ring
                    nc.vector.tensor_scalar(
                        h1_sb[:, hi, :], acc[:],
                        b1_sb[:, hi:hi + 1], 0.0, add_op, max_op)

            # ---- phases 2+3: out = concat @ W2 (bf16), 8 PSUM banks ----
            with (
                tc.tile_pool(name="psacc", bufs=1, space="PSUM") as psacc,
                tc.tile_pool(name="outp", bufs=2) as outp,
            ):
                accs = [[psacc.tile([128, ocs], f32, tag=f"a{oh}_{bt}",
                                    name=f"acc2_{oh}_{bt}")
                         for bt in range(nb)] for oh in (0, 1)]

                def evict_one(acc, bt, oh):
                    out_t = outp.tile([128, ocs], f32)
                    # split across DVE and ACT so evictions drain in
                    # parallel
                    if bt % 2 == 0:
                        nc.vector.tensor_copy(out_t[:], acc[:])
                    else:
                        nc.scalar.activation(
                            out_t[:], acc[:],
                            mybir.ActivationFunctionType.Copy)
                    rings[bt % 2].dma_start(
                        OUTT.ap()[bt * 128:(bt + 1) * 128,
                                  oh * ocs:(oh + 1) * ocs],
                        out_t[:])

                # phase 2: h-part for both output halves
                for oh in (0, 1):
                    for bi, kt0 in enumerate(range(kt1, kt2, kb)):
                        if oh == 0 and bi < n_w2_prefetch:
                            w2_t = w2_pre[bi]
                        else:
                            w2_t = w2p.tile([128, kb, ocs], bf16,
                                            name="w2_t")
                            w2_dma(rings[bi % 2], w2_t, kt0, oh)
                        for j in range(kb):
                            kt = kt0 + j
                            for bt in range(nb):
                                nc.tensor.matmul(
                                    accs[oh][bt][:],
                                    h1_sb[:, kt - kt1,
                                          bt * 128:bt * 128 + 128],
                                    w2_t[:, j, :],
                                    start=(kt == kt1), stop=False)

                # phase 3: x-part.  half 0 fully, evict it (overlaps
                # half 1's matmuls), then half 1 with the last two
                # batches bt-major so evictions overlap the tail.
                for bi, kt0 in enumerate(range(0, kt1, kb)):
                    w2_t = w2p.tile([128, kb, ocs], bf16, name="w2_t")
                    w2_dma(rings[bi % 2], w2_t, kt0, 0)
                    for j in range(kb):
                        kt = kt0 + j
                        for bt in range(nb):
                            nc.tensor.matmul(
                                accs[0][bt][:],
                                xt_sb[:, kt, bt * 128:bt * 128 + 128],
                                w2_t[:, j, :],
                                start=False, stop=(kt == kt1 - 1))
                for bt in range(nb):
                    evict_one(accs[0][bt], bt, 0)

                tail0 = kt1 - 2 * kb
                for bi, kt0 in enumerate(range(0, tail0, kb)):
                    w2_t = w2p.tile([128, kb, ocs], bf16, name="w2_t")
                    w2_dma(rings[bi % 2], w2_t, kt0, 1)
                    for j in range(kb):
                        kt = kt0 + j
                        for bt in range(nb):
                            nc.tensor.matmul(
                                accs[1][bt][:],
                                xt_sb[:, kt, bt * 128:bt * 128 + 128],
                                w2_t[:, j, :],
                                start=False, stop=False)
                w2_ta = w2p.tile([128, kb, ocs], bf16, name="w2_t")
                w2_dma(rings[0], w2_ta, tail0, 1)
                w2_tb = w2p.tile([128, kb, ocs], bf16, name="w2_t")
                w2_dma(rings[1], w2_tb, tail0 + kb, 1)
                for bt in range(nb):
                    for w2x, k0 in ((w2_ta, tail0), (w2_tb, tail0 + kb)):
                        for j in range(kb):
                            kt = k0 + j
                            nc.tensor.matmul(
                                accs[1][bt][:],
                                xt_sb[:, kt, bt * 128:bt * 128 + 128],
                                w2x[:, j, :],
                                start=False, stop=(kt == kt1 - 1))
                    evict_one(accs[1][bt], bt, 1)

    nc.compile()
    return nc


def prep_inputs(x, W1, b1, W2, b2, bc=BC, ktf=KTF):
    """Host-side cast to bf16/fp8 + re-layout so device DMAs are
    contiguous.  Folds the S1 scale: W1,b1 scaled up, W2 h-cols down."""
    d = x.shape[1]
    hid = W1.shape[0]
    out_n = W2.shape[0]
    nh = hid // 128
    kt1 = d // 128
    kt2 = (d + hid) // 128

    w1s = np.asarray(W1, np.float32) * S1
    # [hi, p, kt, h] = S1*W1[hi*128+h, kt*128+p]
    w1_4d = w1s.reshape(nh, 128, kt1, 128).transpose(0, 3, 2, 1)
    w1f = np.ascontiguousarray(w1_4d[:, :, :ktf, :]).astype(nf8)
    w1b = np.ascontiguousarray(w1_4d[:, :, ktf:, :]).astype(nbf) \
        .reshape(nh, 128, (kt1 - ktf) * 128)

    w2s = np.asarray(W2, np.float32).copy()
    w2s[:, d:] /= S1
    w2b = w2s.astype(nbf)
    ocs = out_n // 2
    # W2P[p, kt, o] = W2'[o, kt*128+p]  (partition-major, 4KB lines)
    w2p = w2b.reshape(out_n, kt2, 128).transpose(2, 1, 0)
    w2a = np.ascontiguousarray(w2p[:, :, :ocs])
    w2bb = np.ascontiguousarray(w2p[:, :, ocs:])

    b1r = np.ascontiguousarray(
        (np.asarray(b1, np.float32) * S1).reshape(nh, 128).T)

    xb = np.asarray(x).astype(nbf)
    x8 = np.asarray(x, np.float32).astype(nf8)
    ncores = x.shape[0] // bc
    in_maps = []
    for c in range(ncores):
        # [p, kt, b] partition-major
        xt_c = np.ascontiguousarray(
            xb[c * bc:(c + 1) * bc].T.reshape(kt1, 128, bc)
            .transpose(1, 0, 2))
        xq_c = np.ascontiguousarray(
            x8[c * bc:(c + 1) * bc, :ktf * 128].T.reshape(ktf, 128, bc)
            .transpose(1, 0, 2))
        in_maps.append({"xt": xt_c, "xq": xq_c, "w1f": w1f, "w1b": w1b,
                        "w2a": w2a, "w2b": w2bb, "b1r": b1r})
    return in_maps


def kernel(x, W1, b1, W2, b2):
    x = np.asarray(x)
    W1, b1 = np.asarray(W1), np.asarray(b1)
    W2, b2 = np.asarray(W2), np.asarray(b2)

    if "nc" not in _cache:
        _cache["nc"] = build()
    nc = _cache["nc"]

    in_maps = prep_inputs(x, W1, b1, W2, b2)
    res = run_bass_kernel_spmd(nc, in_maps, core_ids=list(range(NCORES)))
    out = np.concatenate([res.results[c]["out"] for c in range(NCORES)],
                         axis=0)
    return out + np.asarray(b2, np.float32)[None, :]
